# revision 5
# baseline (speedup 1.0000x reference)
"""MHA kernel for TRN2: x[8,512,32,32], 8 heads, S=1024, C=512.

Sharding: data-parallel over batch N=8 -> one batch item per NeuronCore.
Per-core layout (all transpose-free):
  qkT[e,s]  = w_qkvT[:, :1024].T @ x      (e on partitions; q tiles 0-3, k tiles 4-7)
  v[s,e]    = x.T @ w_qkvT[:, 1024:]      (s on partitions, natural layout)
  scoresT   = kT_h.T @ qT_h               (k_s on partitions; K=64 -> head pair packed
                                           at array rows 0-63 / 64-127)
  P         = exp(scoresT * 1/8)          (ACT, batched 2048-wide from PSUM)
  oT_aug    = [v_h | 1].T @ P             (M=65; row 64 = softmax denominator r)
  oT        = oT_aug[:64] * (1/r)         (gpsimd partition_broadcast of 1/r)
  yT[o,s]   = w_outT.T @ oT               (+ b_out added host-side; == NCHW layout)
"""

import numpy as np
import ml_dtypes

import concourse.bacc as bacc
import concourse.mybir as mybir
import concourse.tile as tile
from concourse.bass_utils import run_bass_kernel_spmd

P = 128
S = 1024          # sequence = 32*32
C = 512           # channels
NH = 8            # heads
HD = 64           # head dim
CT = C // P       # 4 c-tiles
ET = 2 * C // P   # 8 e-tiles for q+k
MT = S // P       # 8 s-tiles
BF = mybir.dt.bfloat16
F32 = mybir.dt.float32

_cache = {}


def build_program(dbg=False):
    nc = bacc.Bacc("TRN2", target_bir_lowering=False, debug=False, num_devices=8)
    x_d = nc.dram_tensor("x", [C, S], BF, kind="ExternalInput").ap()
    wq_d = nc.dram_tensor("wq", [C, 3 * C], BF, kind="ExternalInput").ap()
    wo_d = nc.dram_tensor("wo", [C, C], BF, kind="ExternalInput").ap()
    y_d = nc.dram_tensor("y", [C, S], F32, kind="ExternalOutput").ap()
    if dbg:
        dbg_qk0 = nc.dram_tensor("dbg_qk0", [P, S], BF, kind="ExternalOutput").ap()
        dbg_qk4 = nc.dram_tensor("dbg_qk4", [P, S], BF, kind="ExternalOutput").ap()
        dbg_v0 = nc.dram_tensor("dbg_v0", [P, NH * (HD + 1)], BF, kind="ExternalOutput").ap()
        dbg_p00 = nc.dram_tensor("dbg_p00", [P, 2048], BF, kind="ExternalOutput").ap()
        dbg_bc = nc.dram_tensor("dbg_bc", [HD, 512], F32, kind="ExternalOutput").ap()
        dbg_o0 = nc.dram_tensor("dbg_o0", [P, S], BF, kind="ExternalOutput").ap()

    with tile.TileContext(nc) as tc:
        with (
            tc.tile_pool(name="const", bufs=1) as cpool,
            tc.tile_pool(name="qk", bufs=1) as qkpool,
            tc.tile_pool(name="vp", bufs=1) as vpool,
            tc.tile_pool(name="pp", bufs=16) as ppool,
            tc.tile_pool(name="ot", bufs=1) as opool,
            tc.tile_pool(name="yp", bufs=1) as ypool,
            tc.tile_pool(name="misc", bufs=4) as mpool,
            tc.tile_pool(name="psq", bufs=1, space="PSUM") as psq_pool,
            tc.tile_pool(name="pso", bufs=2, space="PSUM") as pso_pool,
            tc.tile_pool(name="psm", bufs=2, space="PSUM") as psm_pool,
        ):
            # ---- load inputs ----
            x_sb, w_sb, wo_sb = [], [], []
            for ct in range(CT):
                xt = cpool.tile([P, S], BF, name=f"x{ct}", tag=f"x{ct}")
                nc.sync.dma_start(xt[:], x_d[ct * P:(ct + 1) * P, :])
                x_sb.append(xt)
                wt = cpool.tile([P, 3 * C], BF, name=f"w{ct}", tag=f"w{ct}")
                nc.sync.dma_start(wt[:], wq_d[ct * P:(ct + 1) * P, :])
                w_sb.append(wt)
            for ct in range(CT):
                wot = cpool.tile([P, C], BF, name=f"wo{ct}", tag=f"wo{ct}")
                nc.sync.dma_start(wot[:], wo_d[ct * P:(ct + 1) * P, :])
                wo_sb.append(wot)

            # ---- qkT projection: [e=1024 rows, s=1024] ----
            qk_sb = []
            for et in range(ET):
                t = qkpool.tile([P, S], BF, name=f"qk{et}", tag=f"qk{et}")
                qk_sb.append(t)
            for et in range(ET):
                for nt in range(2):
                    ps = psm_pool.tile([P, 512], F32, name="psm", tag="psm")
                    for ct in range(CT):
                        nc.tensor.matmul(
                            ps[:],
                            w_sb[ct][:, et * P:(et + 1) * P],
                            x_sb[ct][:, nt * 512:(nt + 1) * 512],
                            start=(ct == 0), stop=(ct == CT - 1),
                        )
                    dst = qk_sb[et][:, nt * 512:(nt + 1) * 512]
                    if et % 2 == 0:
                        nc.vector.tensor_copy(dst, ps[:])
                    else:
                        nc.scalar.copy(dst, ps[:])

            # ---- v projection, natural layout + ones col: [s, 8*(64+1)] ----
            v_sb = []
            for mt in range(MT):
                vt = vpool.tile([P, NH * (HD + 1)], BF, name=f"v{mt}", tag=f"v{mt}")
                nc.vector.memset(vt[:], 1.0)
                ps = psm_pool.tile([P, 512], F32, name="psm", tag="psm")
                for ct in range(CT):
                    nc.tensor.matmul(
                        ps[:],
                        x_sb[ct][:, mt * P:(mt + 1) * P],
                        w_sb[ct][:, 2 * C:3 * C],
                        start=(ct == 0), stop=(ct == CT - 1),
                    )
                dst = vt[:].rearrange("p (h e) -> p h e", e=HD + 1)[:, :, 0:HD]
                src = ps[:].rearrange("p (h e) -> p h e", e=HD)
                nc.vector.tensor_copy(dst, src)
                v_sb.append(vt)
                if dbg and mt == 0:
                    nc.sync.dma_start(dbg_v0[:], vt[:])

            # ---- attention, head pair p = heads (2p, 2p+1) ----
            oT_sb = [opool.tile([P, S], BF, name=f"o{ct}", tag=f"o{ct}") for ct in range(CT)]
            for p in range(NH // 2):
                p_tiles = []
                for mt in range(MT):
                    psq = psq_pool.tile([P, 2048], F32, name="psq", tag="psq")
                    for nt in range(2):
                        for hh in range(2):
                            # kT_h as lhsT (K=64 at rows hh*64), qT_h as rhs
                            nc.tensor.matmul(
                                psq[:, (2 * nt + hh) * 512:(2 * nt + hh + 1) * 512],
                                qk_sb[4 + p][hh * HD:(hh + 1) * HD, mt * P:(mt + 1) * P],
                                qk_sb[p][hh * HD:(hh + 1) * HD, nt * 512:(nt + 1) * 512],
                                start=True, stop=True,
                            )
                    pt = ppool.tile([P, 2048], BF, name="ptile", tag="ptile")
                    nc.scalar.activation(
                        pt[:], psq[:], mybir.ActivationFunctionType.Exp,
                        scale=float(1.0 / np.sqrt(HD)),
                    )
                    p_tiles.append(pt)
                    if dbg and p == 0 and mt == 0:
                        nc.sync.dma_start(dbg_p00[:], pt[:])

                for hh in range(2):
                    h = 2 * p + hh
                    ct, half = h // 2, h % 2
                    for nt in range(2):
                        pso = pso_pool.tile([P, 512], F32, name="pso", tag="pso")
                        for mt in range(MT):
                            nc.tensor.matmul(
                                pso[0:HD + 1, :],
                                v_sb[mt][:, h * (HD + 1):(h + 1) * (HD + 1)],
                                p_tiles[mt][:, (2 * nt + hh) * 512:(2 * nt + hh + 1) * 512],
                                start=(mt == 0), stop=(mt == MT - 1),
                            )
                        rinv = mpool.tile([1, 512], F32, name="rinv", tag="rinv")
                        nc.vector.reciprocal(rinv[0:1, :], pso[HD:HD + 1, :])
                        bc = mpool.tile([HD, 512], F32, name="bc", tag="bc")
                        nc.gpsimd.partition_broadcast(bc[:], rinv[0:1, :], channels=HD)
                        if dbg and h == 0 and nt == 0:
                            nc.sync.dma_start(dbg_bc[:], bc[:])
                        nc.vector.tensor_mul(
                            oT_sb[ct][half * HD:(half + 1) * HD, nt * 512:(nt + 1) * 512],
                            pso[0:HD, :], bc[:],
                        )

            if dbg:
                nc.sync.dma_start(dbg_qk0[:], qk_sb[0][:])
                nc.sync.dma_start(dbg_qk4[:], qk_sb[4][:])
                nc.sync.dma_start(dbg_o0[:], oT_sb[0][:])
            # ---- output projection: yT[o, s] ----
            for ot in range(CT):
                yt = ypool.tile([P, S], F32, name=f"y{ot}", tag=f"y{ot}")
                for st_ in range(2):
                    ps = psm_pool.tile([P, 512], F32, name="psm", tag="psm")
                    for ct in range(CT):
                        nc.tensor.matmul(
                            ps[:],
                            wo_sb[ct][:, ot * P:(ot + 1) * P],
                            oT_sb[ct][:, st_ * 512:(st_ + 1) * 512],
                            start=(ct == 0), stop=(ct == CT - 1),
                        )
                    dst = yt[:, st_ * 512:(st_ + 1) * 512]
                    if st_ == 0:
                        nc.scalar.copy(dst, ps[:])
                    else:
                        nc.vector.tensor_copy(dst, ps[:])
                nc.sync.dma_start(y_d[ot * P:(ot + 1) * P, :], yt[:])

    nc.compile()
    return nc


def get_program():
    if "nc" not in _cache:
        _cache["nc"] = build_program()
    return _cache["nc"]


def kernel(x, w_qkv, w_out, b_out, _trace=False, _tmpdir=None):
    x = np.asarray(x, dtype=np.float32)
    w_qkv = np.asarray(w_qkv, dtype=np.float32)
    w_out = np.asarray(w_out, dtype=np.float32)
    b_out = np.asarray(b_out, dtype=np.float32)
    N = x.shape[0]

    xb = x.reshape(N, C, S).astype(ml_dtypes.bfloat16)
    wqT = np.ascontiguousarray(w_qkv.T).astype(ml_dtypes.bfloat16)
    woT = np.ascontiguousarray(w_out.T).astype(ml_dtypes.bfloat16)

    nc = get_program()
    in_maps = [
        {"x": np.ascontiguousarray(xb[n]), "wq": wqT, "wo": woT}
        for n in range(N)
    ]
    res = run_bass_kernel_spmd(
        nc, in_maps, core_ids=list(range(N)), trace=_trace, tmpdir=_tmpdir
    )
    y = np.stack([res.results[n]["y"] for n in range(N)])
    y = y.reshape(N, C, 32, 32).astype(np.float32)
    y = y + b_out[None, :, None, None]
    if _trace:
        return y, res
    return y


# revision 6
# speedup vs baseline: 1.1932x; 1.1932x over previous
"""MHA kernel for TRN2: x[8,512,32,32], 8 heads, S=1024, C=512.

Sharding: data-parallel over batch N=8 -> one batch item per NeuronCore.
Per-core layout (all transpose-free):
  qkT[e,s]  = w_qkvT[:, :1024].T @ x      (e on partitions; q tiles 0-3, k tiles 4-7)
  v[s,e]    = x.T @ w_qkvT[:, 1024:]      (s on partitions, natural layout)
  scoresT   = kT_h.T @ qT_h               (k_s on partitions; K=64 -> head pair packed
                                           at array rows 0-63 / 64-127)
  P         = exp(scoresT * 1/8)          (ACT, batched 2048-wide from PSUM)
  oT_aug    = [v_h | 1].T @ P             (M=65; row 64 = softmax denominator r)
  oT        = oT_aug[:64] * (1/r)         (gpsimd partition_broadcast of 1/r)
  yT[o,s]   = w_outT.T @ oT               (+ b_out added host-side; == NCHW layout)
"""

import numpy as np
import ml_dtypes

import concourse.bacc as bacc
import concourse.mybir as mybir
import concourse.tile as tile
from concourse.bass_utils import run_bass_kernel_spmd

P = 128
S = 1024          # sequence = 32*32
C = 512           # channels
NH = 8            # heads
HD = 64           # head dim
CT = C // P       # 4 c-tiles
ET = 2 * C // P   # 8 e-tiles for q+k
MT = S // P       # 8 s-tiles
BF = mybir.dt.bfloat16
F32 = mybir.dt.float32

_cache = {}


def build_program(dbg=False):
    nc = bacc.Bacc("TRN2", target_bir_lowering=False, debug=False, num_devices=8)
    x_d = nc.dram_tensor("x", [C, S], BF, kind="ExternalInput").ap()
    wq_d = nc.dram_tensor("wq", [C, 3 * C], BF, kind="ExternalInput").ap()
    wo_d = nc.dram_tensor("wo", [C, C], BF, kind="ExternalInput").ap()
    y_d = nc.dram_tensor("y", [C, S], F32, kind="ExternalOutput").ap()
    if dbg:
        dbg_qk0 = nc.dram_tensor("dbg_qk0", [P, S], BF, kind="ExternalOutput").ap()
        dbg_qk4 = nc.dram_tensor("dbg_qk4", [P, S], BF, kind="ExternalOutput").ap()
        dbg_v0 = nc.dram_tensor("dbg_v0", [P, NH * (HD + 1)], BF, kind="ExternalOutput").ap()
        dbg_p00 = nc.dram_tensor("dbg_p00", [P, 2048], BF, kind="ExternalOutput").ap()
        dbg_bc = nc.dram_tensor("dbg_bc", [HD, 512], F32, kind="ExternalOutput").ap()
        dbg_o0 = nc.dram_tensor("dbg_o0", [P, S], BF, kind="ExternalOutput").ap()

    with tile.TileContext(nc) as tc:
        with (
            tc.tile_pool(name="const", bufs=1) as cpool,
            tc.tile_pool(name="qk", bufs=1) as qkpool,
            tc.tile_pool(name="vp", bufs=1) as vpool,
            tc.tile_pool(name="pp", bufs=16) as ppool,
            tc.tile_pool(name="ot", bufs=1) as opool,
            tc.tile_pool(name="yp", bufs=1) as ypool,
            tc.tile_pool(name="misc", bufs=4) as mpool,
            tc.tile_pool(name="psq", bufs=1, space="PSUM") as psq_pool,
            tc.tile_pool(name="pso", bufs=4, space="PSUM") as pso_pool,
        ):
            # ---- load inputs ----
            x_sb, w_sb, wo_sb = [], [], []
            for ct in range(CT):
                xt = cpool.tile([P, S], BF, name=f"x{ct}", tag=f"x{ct}")
                nc.sync.dma_start(xt[:], x_d[ct * P:(ct + 1) * P, :])
                x_sb.append(xt)
                wt = cpool.tile([P, 3 * C], BF, name=f"w{ct}", tag=f"w{ct}")
                nc.sync.dma_start(wt[:], wq_d[ct * P:(ct + 1) * P, :])
                w_sb.append(wt)
            for ct in range(CT):
                wot = cpool.tile([P, C], BF, name=f"wo{ct}", tag=f"wo{ct}")
                nc.sync.dma_start(wot[:], wo_d[ct * P:(ct + 1) * P, :])
                wo_sb.append(wot)

            # ---- qkT projection: [e=1024 rows, s=1024] ----
            qk_sb = []
            for et in range(ET):
                t = qkpool.tile([P, S], BF, name=f"qk{et}", tag=f"qk{et}")
                qk_sb.append(t)
            groups = [(et, nt) for et in range(ET) for nt in range(2)]
            for bk in range(0, len(groups), 4):
                blk = groups[bk:bk + 4]
                pss = [pso_pool.tile([P, 512], F32, name=f"qp{i}", tag="pso")
                       for i in range(len(blk))]
                for ct in range(CT):
                    for g, (et, nt) in enumerate(blk):
                        nc.tensor.matmul(
                            pss[g][:],
                            w_sb[ct][:, et * P:(et + 1) * P],
                            x_sb[ct][:, nt * 512:(nt + 1) * 512],
                            start=(ct == 0), stop=(ct == CT - 1),
                        )
                for g, (et, nt) in enumerate(blk):
                    dst = qk_sb[et][:, nt * 512:(nt + 1) * 512]
                    if g % 2 == 0:
                        nc.vector.tensor_copy(dst, pss[g][:])
                    else:
                        nc.scalar.copy(dst, pss[g][:])

            # ---- v projection, natural layout + ones col: [s, 8*(64+1)] ----
            v_sb = [None] * MT
            for bk in range(0, MT, 4):
                pss = [pso_pool.tile([P, 512], F32, name=f"vp{i}", tag="pso")
                       for i in range(4)]
                for ct in range(CT):
                    for g in range(4):
                        mt = bk + g
                        nc.tensor.matmul(
                            pss[g][:],
                            x_sb[ct][:, mt * P:(mt + 1) * P],
                            w_sb[ct][:, 2 * C:3 * C],
                            start=(ct == 0), stop=(ct == CT - 1),
                        )
                for g in range(4):
                    mt = bk + g
                    vt = vpool.tile([P, NH * (HD + 1)], BF, name=f"v{mt}", tag=f"v{mt}")
                    nc.vector.memset(vt[:], 1.0)
                    dst = vt[:].rearrange("p (h e) -> p h e", e=HD + 1)[:, :, 0:HD]
                    srcp = pss[g][:].rearrange("p (h e) -> p h e", e=HD)
                    if g % 2 == 0:
                        nc.vector.tensor_copy(dst, srcp)
                    else:
                        nc.scalar.copy(dst, srcp)
                    v_sb[mt] = vt

            # ---- attention, software-pipelined: QK/exp(pair p) || PV(pair p-1) ----
            oT_sb = [opool.tile([P, S], BF, name=f"o{ct}", tag=f"o{ct}") for ct in range(CT)]
            p_tiles = {}
            DRAIN_ORDER = ((0, 0), (1, 0), (0, 1), (1, 1))
            for step in range(NH // 2 + 1):
                pso_t = None
                if step >= 1:
                    pso_t = [pso_pool.tile([P, 512], F32, name=f"pso{i}", tag="pso")
                             for i in range(4)]
                for mt in range(MT):
                    if step < NH // 2:
                        psq = psq_pool.tile([P, 2048], F32, name="psq", tag="psq")
                        for nt in range(2):
                            for hh in range(2):
                                nc.tensor.matmul(
                                    psq[:, (2 * nt + hh) * 512:(2 * nt + hh + 1) * 512],
                                    qk_sb[4 + step][hh * HD:(hh + 1) * HD, mt * P:(mt + 1) * P],
                                    qk_sb[step][hh * HD:(hh + 1) * HD, nt * 512:(nt + 1) * 512],
                                    start=True, stop=True,
                                )
                        pt = ppool.tile([P, 2048], BF, name="ptile", tag="ptile")
                        nc.scalar.activation(
                            pt[:], psq[:], mybir.ActivationFunctionType.Exp,
                            scale=float(1.0 / np.sqrt(HD)),
                        )
                        p_tiles[(step, mt)] = pt
                    if step >= 1:
                        pp = step - 1
                        for idx, (hh, nt) in enumerate(DRAIN_ORDER):
                            h = 2 * pp + hh
                            nc.tensor.matmul(
                                pso_t[idx][0:HD + 1, :],
                                v_sb[mt][:, h * (HD + 1):(h + 1) * (HD + 1)],
                                p_tiles[(pp, mt)][:, (2 * nt + hh) * 512:(2 * nt + hh + 1) * 512],
                                start=(mt == 0), stop=(mt == MT - 1),
                            )
                if step >= 1:
                    pp = step - 1
                    for idx, (hh, nt) in enumerate(DRAIN_ORDER):
                        h = 2 * pp + hh
                        ct, half = h // 2, h % 2
                        rinv = mpool.tile([1, 512], F32, name="rinv", tag="rinv")
                        nc.vector.reciprocal(rinv[0:1, :], pso_t[idx][HD:HD + 1, :])
                        bc = mpool.tile([HD, 512], F32, name="bc", tag="bc")
                        nc.gpsimd.partition_broadcast(bc[:], rinv[0:1, :], channels=HD)
                        nc.vector.tensor_mul(
                            oT_sb[ct][half * HD:(half + 1) * HD, nt * 512:(nt + 1) * 512],
                            pso_t[idx][0:HD, :], bc[:],
                        )

            # ---- output projection: yT[o, s] ----
            y_sb = [ypool.tile([P, S], F32, name=f"y{ot}", tag=f"y{ot}") for ot in range(CT)]
            og = [(ot, st_) for ot in range(CT) for st_ in range(2)]
            for bk in range(0, len(og), 4):
                blk = og[bk:bk + 4]
                pss = [pso_pool.tile([P, 512], F32, name=f"op{i}", tag="pso")
                       for i in range(len(blk))]
                for ct in range(CT):
                    for g, (ot, st_) in enumerate(blk):
                        nc.tensor.matmul(
                            pss[g][:],
                            wo_sb[ct][:, ot * P:(ot + 1) * P],
                            oT_sb[ct][:, st_ * 512:(st_ + 1) * 512],
                            start=(ct == 0), stop=(ct == CT - 1),
                        )
                for g, (ot, st_) in enumerate(blk):
                    dst = y_sb[ot][:, st_ * 512:(st_ + 1) * 512]
                    if g % 2 == 0:
                        nc.scalar.copy(dst, pss[g][:])
                    else:
                        nc.vector.tensor_copy(dst, pss[g][:])
            for ot in range(CT):
                nc.sync.dma_start(y_d[ot * P:(ot + 1) * P, :], y_sb[ot][:])

    nc.compile()
    return nc


def get_program():
    if "nc" not in _cache:
        _cache["nc"] = build_program()
    return _cache["nc"]


def kernel(x, w_qkv, w_out, b_out, _trace=False, _tmpdir=None):
    x = np.asarray(x, dtype=np.float32)
    w_qkv = np.asarray(w_qkv, dtype=np.float32)
    w_out = np.asarray(w_out, dtype=np.float32)
    b_out = np.asarray(b_out, dtype=np.float32)
    N = x.shape[0]

    xb = x.reshape(N, C, S).astype(ml_dtypes.bfloat16)
    wqT = np.ascontiguousarray(w_qkv.T).astype(ml_dtypes.bfloat16)
    woT = np.ascontiguousarray(w_out.T).astype(ml_dtypes.bfloat16)

    nc = get_program()
    in_maps = [
        {"x": np.ascontiguousarray(xb[n]), "wq": wqT, "wo": woT}
        for n in range(N)
    ]
    res = run_bass_kernel_spmd(
        nc, in_maps, core_ids=list(range(N)), trace=_trace, tmpdir=_tmpdir
    )
    y = np.stack([res.results[n]["y"] for n in range(N)])
    y = y.reshape(N, C, 32, 32).astype(np.float32)
    y = y + b_out[None, :, None, None]
    if _trace:
        return y, res
    return y


# revision 8
# speedup vs baseline: 1.3274x; 1.1125x over previous
"""MHA kernel for TRN2: x[8,512,32,32], 8 heads, S=1024, C=512.

Sharding: data-parallel over batch N=8 -> one batch item per NeuronCore.
Per-core layout (all transpose-free):
  qkT[e,s]  = w_qkvT[:, :1024].T @ x      (e on partitions; q tiles 0-3, k tiles 4-7)
  v[s,e]    = x.T @ w_qkvT[:, 1024:]      (s on partitions, natural layout)
  scoresT   = kT_h.T @ qT_h               (k_s on partitions; K=64 -> head pair packed
                                           at array rows 0-63 / 64-127)
  P         = exp(scoresT * 1/8)          (ACT, batched 2048-wide from PSUM)
  oT_aug    = [v_h | 1].T @ P             (M=65; row 64 = softmax denominator r)
  oT        = oT_aug[:64] * (1/r)         (gpsimd partition_broadcast of 1/r)
  yT[o,s]   = w_outT.T @ oT               (+ b_out added host-side; == NCHW layout)
"""

import numpy as np
import ml_dtypes

import concourse.bacc as bacc
import concourse.mybir as mybir
import concourse.tile as tile
from concourse.bass_utils import run_bass_kernel_spmd

P = 128
S = 1024          # sequence = 32*32
C = 512           # channels
NH = 8            # heads
HD = 64           # head dim
CT = C // P       # 4 c-tiles
ET = 2 * C // P   # 8 e-tiles for q+k
MT = S // P       # 8 s-tiles
BF = mybir.dt.bfloat16
F32 = mybir.dt.float32

_cache = {}


def build_program(dbg=False):
    nc = bacc.Bacc("TRN2", target_bir_lowering=False, debug=False, num_devices=8)
    x_d = nc.dram_tensor("x", [C, S], BF, kind="ExternalInput").ap()
    wq_d = nc.dram_tensor("wq", [C, 3 * C], BF, kind="ExternalInput").ap()
    wo_d = nc.dram_tensor("wo", [C, C], BF, kind="ExternalInput").ap()
    y_d = nc.dram_tensor("y", [C, S], F32, kind="ExternalOutput").ap()
    if dbg:
        dbg_qk0 = nc.dram_tensor("dbg_qk0", [P, S], BF, kind="ExternalOutput").ap()
        dbg_qk4 = nc.dram_tensor("dbg_qk4", [P, S], BF, kind="ExternalOutput").ap()
        dbg_v0 = nc.dram_tensor("dbg_v0", [P, NH * (HD + 1)], BF, kind="ExternalOutput").ap()
        dbg_p00 = nc.dram_tensor("dbg_p00", [P, 2048], BF, kind="ExternalOutput").ap()
        dbg_bc = nc.dram_tensor("dbg_bc", [HD, 512], F32, kind="ExternalOutput").ap()
        dbg_o0 = nc.dram_tensor("dbg_o0", [P, S], BF, kind="ExternalOutput").ap()

    with tile.TileContext(nc) as tc:
        with (
            tc.tile_pool(name="const", bufs=1) as cpool,
            tc.tile_pool(name="qk", bufs=1) as qkpool,
            tc.tile_pool(name="vp", bufs=1) as vpool,
            tc.tile_pool(name="pp", bufs=32) as ppool,
            tc.tile_pool(name="ot", bufs=1) as opool,
            tc.tile_pool(name="yp", bufs=1) as ypool,
            tc.tile_pool(name="misc", bufs=4) as mpool,
            tc.tile_pool(name="psq", bufs=2, space="PSUM") as psq_pool,
            tc.tile_pool(name="pso", bufs=4, space="PSUM") as pso_pool,
        ):
            # ---- load inputs ----
            x_sb, w_sb, wo_sb = [], [], []
            for ct in range(CT):
                xt = cpool.tile([P, S], BF, name=f"x{ct}", tag=f"x{ct}")
                nc.sync.dma_start(xt[:], x_d[ct * P:(ct + 1) * P, :])
                x_sb.append(xt)
                wt = cpool.tile([P, 3 * C], BF, name=f"w{ct}", tag=f"w{ct}")
                nc.sync.dma_start(wt[:], wq_d[ct * P:(ct + 1) * P, :])
                w_sb.append(wt)
            for ct in range(CT):
                wot = cpool.tile([P, C], BF, name=f"wo{ct}", tag=f"wo{ct}")
                nc.sync.dma_start(wot[:], wo_d[ct * P:(ct + 1) * P, :])
                wo_sb.append(wot)

            # ---- qkT projection: [e=1024 rows, s=1024] ----
            qk_sb = []
            for et in range(ET):
                t = qkpool.tile([P, S], BF, name=f"qk{et}", tag=f"qk{et}")
                qk_sb.append(t)
            groups = [(et, nt) for et in range(ET) for nt in range(2)]
            for bk in range(0, len(groups), 4):
                blk = groups[bk:bk + 4]
                pss = [pso_pool.tile([P, 512], F32, name=f"qp{i}", tag="pso")
                       for i in range(len(blk))]
                for ct in range(CT):
                    for g, (et, nt) in enumerate(blk):
                        nc.tensor.matmul(
                            pss[g][:],
                            w_sb[ct][:, et * P:(et + 1) * P],
                            x_sb[ct][:, nt * 512:(nt + 1) * 512],
                            start=(ct == 0), stop=(ct == CT - 1),
                        )
                for g, (et, nt) in enumerate(blk):
                    dst = qk_sb[et][:, nt * 512:(nt + 1) * 512]
                    if g % 2 == 0:
                        nc.vector.tensor_copy(dst, pss[g][:])
                    else:
                        nc.scalar.copy(dst, pss[g][:])

            # ---- v projection, natural layout + ones col: [s, 8*(64+1)] ----
            v_sb = [None] * MT
            for bk in range(0, MT, 4):
                pss = [pso_pool.tile([P, 512], F32, name=f"vp{i}", tag="pso")
                       for i in range(4)]
                for ct in range(CT):
                    for g in range(4):
                        mt = bk + g
                        nc.tensor.matmul(
                            pss[g][:],
                            x_sb[ct][:, mt * P:(mt + 1) * P],
                            w_sb[ct][:, 2 * C:3 * C],
                            start=(ct == 0), stop=(ct == CT - 1),
                        )
                for g in range(4):
                    mt = bk + g
                    vt = vpool.tile([P, NH * (HD + 1)], BF, name=f"v{mt}", tag=f"v{mt}")
                    nc.vector.memset(vt[:], 1.0)
                    dst = vt[:].rearrange("p (h e) -> p h e", e=HD + 1)[:, :, 0:HD]
                    srcp = pss[g][:].rearrange("p (h e) -> p h e", e=HD)
                    if g % 2 == 0:
                        nc.vector.tensor_copy(dst, srcp)
                    else:
                        nc.scalar.copy(dst, srcp)
                    v_sb[mt] = vt

            # ---- attention, software-pipelined: QK/exp(pair p) || PV(pair p-1) ----
            oT_sb = [opool.tile([P, S], BF, name=f"o{ct}", tag=f"o{ct}") for ct in range(CT)]
            p_tiles = {}
            DRAIN_ORDER = ((0, 0), (1, 0), (0, 1), (1, 1))
            for step in range(NH // 2 + 1):
                pso_t = None
                if step >= 1:
                    pso_t = [pso_pool.tile([P, 512], F32, name=f"pso{i}", tag="pso")
                             for i in range(4)]
                for mt in range(MT):
                    if step < NH // 2:
                        for nt in range(2):
                            psq = psq_pool.tile([P, 1024], F32, name="psq", tag="psq")
                            for hh in range(2):
                                nc.tensor.matmul(
                                    psq[:, hh * 512:(hh + 1) * 512],
                                    qk_sb[4 + step][hh * HD:(hh + 1) * HD, mt * P:(mt + 1) * P],
                                    qk_sb[step][hh * HD:(hh + 1) * HD, nt * 512:(nt + 1) * 512],
                                    start=True, stop=True,
                                )
                            pt = ppool.tile([P, 1024], BF, name="ptile", tag="ptile")
                            nc.scalar.activation(
                                pt[:], psq[:], mybir.ActivationFunctionType.Exp,
                                scale=float(1.0 / np.sqrt(HD)),
                            )
                            p_tiles[(step, mt, nt)] = pt
                    if step >= 1:
                        pp = step - 1
                        for idx, (hh, nt) in enumerate(DRAIN_ORDER):
                            h = 2 * pp + hh
                            nc.tensor.matmul(
                                pso_t[idx][0:HD + 1, :],
                                v_sb[mt][:, h * (HD + 1):(h + 1) * (HD + 1)],
                                p_tiles[(pp, mt, nt)][:, hh * 512:(hh + 1) * 512],
                                start=(mt == 0), stop=(mt == MT - 1),
                            )
                if step >= 1:
                    pp = step - 1
                    for idx, (hh, nt) in enumerate(DRAIN_ORDER):
                        h = 2 * pp + hh
                        ct, half = h // 2, h % 2
                        rinv = mpool.tile([1, 512], F32, name="rinv", tag="rinv")
                        nc.vector.reciprocal(rinv[0:1, :], pso_t[idx][HD:HD + 1, :])
                        bc = mpool.tile([HD, 512], F32, name="bc", tag="bc")
                        nc.gpsimd.partition_broadcast(bc[:], rinv[0:1, :], channels=HD)
                        nc.vector.tensor_mul(
                            oT_sb[ct][half * HD:(half + 1) * HD, nt * 512:(nt + 1) * 512],
                            pso_t[idx][0:HD, :], bc[:],
                        )

            # ---- output projection: yT[o, s] ----
            y_sb = [ypool.tile([P, S], F32, name=f"y{ot}", tag=f"y{ot}") for ot in range(CT)]
            og = [(ot, st_) for ot in range(CT) for st_ in range(2)]
            for bk in range(0, len(og), 4):
                blk = og[bk:bk + 4]
                pss = [pso_pool.tile([P, 512], F32, name=f"op{i}", tag="pso")
                       for i in range(len(blk))]
                for ct in range(CT):
                    for g, (ot, st_) in enumerate(blk):
                        nc.tensor.matmul(
                            pss[g][:],
                            wo_sb[ct][:, ot * P:(ot + 1) * P],
                            oT_sb[ct][:, st_ * 512:(st_ + 1) * 512],
                            start=(ct == 0), stop=(ct == CT - 1),
                        )
                for g, (ot, st_) in enumerate(blk):
                    dst = y_sb[ot][:, st_ * 512:(st_ + 1) * 512]
                    if g % 2 == 0:
                        nc.scalar.copy(dst, pss[g][:])
                    else:
                        nc.vector.tensor_copy(dst, pss[g][:])
            for ot in range(CT):
                nc.sync.dma_start(y_d[ot * P:(ot + 1) * P, :], y_sb[ot][:])

    nc.compile()
    return nc


def get_program():
    if "nc" not in _cache:
        _cache["nc"] = build_program()
    return _cache["nc"]


def kernel(x, w_qkv, w_out, b_out, _trace=False, _tmpdir=None):
    x = np.asarray(x, dtype=np.float32)
    w_qkv = np.asarray(w_qkv, dtype=np.float32)
    w_out = np.asarray(w_out, dtype=np.float32)
    b_out = np.asarray(b_out, dtype=np.float32)
    N = x.shape[0]

    xb = x.reshape(N, C, S).astype(ml_dtypes.bfloat16)
    wqT = np.ascontiguousarray(w_qkv.T).astype(ml_dtypes.bfloat16)
    woT = np.ascontiguousarray(w_out.T).astype(ml_dtypes.bfloat16)

    nc = get_program()
    in_maps = [
        {"x": np.ascontiguousarray(xb[n]), "wq": wqT, "wo": woT}
        for n in range(N)
    ]
    res = run_bass_kernel_spmd(
        nc, in_maps, core_ids=list(range(N)), trace=_trace, tmpdir=_tmpdir
    )
    y = np.stack([res.results[n]["y"] for n in range(N)])
    y = y.reshape(N, C, 32, 32).astype(np.float32)
    y = y + b_out[None, :, None, None]
    if _trace:
        return y, res
    return y


# revision 9
# speedup vs baseline: 1.7499x; 1.3183x over previous
"""MHA kernel for TRN2: x[8,512,32,32], 8 heads, S=1024, C=512.

Sharding: data-parallel over batch N=8 -> one batch item per NeuronCore.
Per-core layout (all transpose-free):
  qkT[e,s]  = w_qkvT[:, :1024].T @ x      (e on partitions; q tiles 0-3, k tiles 4-7)
  v[s,e]    = x.T @ w_qkvT[:, 1024:]      (s on partitions, natural layout)
  scoresT   = kT_h.T @ qT_h               (k_s on partitions; K=64 -> head pair packed
                                           at array rows 0-63 / 64-127)
  P         = exp(scoresT * 1/8)          (ACT, batched 2048-wide from PSUM)
  oT_aug    = [v_h | 1].T @ P             (M=65; row 64 = softmax denominator r)
  oT        = oT_aug[:64] * (1/r)         (gpsimd partition_broadcast of 1/r)
  yT[o,s]   = w_outT.T @ oT               (+ b_out added host-side; == NCHW layout)
"""

import numpy as np
import ml_dtypes

import concourse.bacc as bacc
import concourse.mybir as mybir
import concourse.tile as tile
from concourse.bass_utils import run_bass_kernel_spmd

P = 128
S = 1024          # sequence = 32*32
C = 512           # channels
NH = 8            # heads
HD = 64           # head dim
CT = C // P       # 4 c-tiles
ET = 2 * C // P   # 8 e-tiles for q+k
MT = S // P       # 8 s-tiles
BF = mybir.dt.bfloat16
F32 = mybir.dt.float32

_cache = {}


def build_program(dbg=False):
    nc = bacc.Bacc("TRN2", target_bir_lowering=False, debug=False, num_devices=8)
    x_d = nc.dram_tensor("x", [C, S], BF, kind="ExternalInput").ap()
    wq_d = nc.dram_tensor("wq", [C, 3 * C], BF, kind="ExternalInput").ap()
    wo_d = nc.dram_tensor("wo", [C, C], BF, kind="ExternalInput").ap()
    y_d = nc.dram_tensor("y", [C, S], F32, kind="ExternalOutput").ap()
    if dbg:
        dbg_qk0 = nc.dram_tensor("dbg_qk0", [P, S], BF, kind="ExternalOutput").ap()
        dbg_qk4 = nc.dram_tensor("dbg_qk4", [P, S], BF, kind="ExternalOutput").ap()
        dbg_v0 = nc.dram_tensor("dbg_v0", [P, NH * (HD + 1)], BF, kind="ExternalOutput").ap()
        dbg_p00 = nc.dram_tensor("dbg_p00", [P, 2048], BF, kind="ExternalOutput").ap()
        dbg_bc = nc.dram_tensor("dbg_bc", [HD, 512], F32, kind="ExternalOutput").ap()
        dbg_o0 = nc.dram_tensor("dbg_o0", [P, S], BF, kind="ExternalOutput").ap()

    with tile.TileContext(nc) as tc:
        with (
            tc.tile_pool(name="const", bufs=1) as cpool,
            tc.tile_pool(name="qk", bufs=1) as qkpool,
            tc.tile_pool(name="vp", bufs=1) as vpool,
            tc.tile_pool(name="pp", bufs=32) as ppool,
            tc.tile_pool(name="ot", bufs=1) as opool,
            tc.tile_pool(name="yp", bufs=1) as ypool,
            tc.tile_pool(name="misc", bufs=4) as mpool,
            tc.tile_pool(name="psq", bufs=2, space="PSUM") as psq_pool,
            tc.tile_pool(name="pso", bufs=4, space="PSUM") as pso_pool,
        ):
            # ---- load inputs ----
            x_sb, w_sb, wo_sb = [], [], []
            for ct in range(CT):
                xt = cpool.tile([P, S], BF, name=f"x{ct}", tag=f"x{ct}")
                nc.sync.dma_start(xt[:], x_d[ct * P:(ct + 1) * P, :])
                x_sb.append(xt)
                wt = cpool.tile([P, 3 * C], BF, name=f"w{ct}", tag=f"w{ct}")
                nc.sync.dma_start(wt[:], wq_d[ct * P:(ct + 1) * P, :])
                w_sb.append(wt)
            for ct in range(CT):
                wot = cpool.tile([P, C], BF, name=f"wo{ct}", tag=f"wo{ct}")
                nc.sync.dma_start(wot[:], wo_d[ct * P:(ct + 1) * P, :])
                wo_sb.append(wot)

            # ---- qkT projection: [e=1024 rows, s=1024] ----
            qk_sb = []
            for et in range(ET):
                t = qkpool.tile([P, S], BF, name=f"qk{et}", tag=f"qk{et}")
                qk_sb.append(t)
            groups = [(et, nt) for et in range(ET) for nt in range(2)]
            for bk in range(0, len(groups), 4):
                blk = groups[bk:bk + 4]
                pss = [pso_pool.tile([P, 512], F32, name=f"qp{i}", tag="pso")
                       for i in range(len(blk))]
                for ct in range(CT):
                    for g, (et, nt) in enumerate(blk):
                        nc.tensor.matmul(
                            pss[g][:],
                            w_sb[ct][:, et * P:(et + 1) * P],
                            x_sb[ct][:, nt * 512:(nt + 1) * 512],
                            start=(ct == 0), stop=(ct == CT - 1),
                        )
                for g, (et, nt) in enumerate(blk):
                    dst = qk_sb[et][:, nt * 512:(nt + 1) * 512]
                    if g % 2 == 0:
                        nc.vector.tensor_copy(dst, pss[g][:])
                    else:
                        nc.scalar.copy(dst, pss[g][:])

            # ---- v projection, natural layout + ones col: [s, 8*(64+1)] ----
            v_sb = [None] * MT
            for bk in range(0, MT, 4):
                pss = [pso_pool.tile([P, 512], F32, name=f"vp{i}", tag="pso")
                       for i in range(4)]
                for ct in range(CT):
                    for g in range(4):
                        mt = bk + g
                        nc.tensor.matmul(
                            pss[g][:],
                            x_sb[ct][:, mt * P:(mt + 1) * P],
                            w_sb[ct][:, 2 * C:3 * C],
                            start=(ct == 0), stop=(ct == CT - 1),
                        )
                for g in range(4):
                    mt = bk + g
                    vt = vpool.tile([P, NH * (HD + 1)], BF, name=f"v{mt}", tag=f"v{mt}")
                    nc.vector.memset(vt[:], 1.0)
                    dst = vt[:].rearrange("p (h e) -> p h e", e=HD + 1)[:, :, 0:HD]
                    srcp = pss[g][:].rearrange("p (h e) -> p h e", e=HD)
                    if g % 2 == 0:
                        nc.vector.tensor_copy(dst, srcp)
                    else:
                        nc.scalar.copy(dst, srcp)
                    v_sb[mt] = vt

            # ---- attention, software-pipelined: QK/exp(pair p) || PV(pair p-1) ----
            oT_sb = [opool.tile([P, S], BF, name=f"o{ct}", tag=f"o{ct}") for ct in range(CT)]
            p_tiles = {}
            DRAIN_ORDER = ((0, 0), (1, 0), (0, 1), (1, 1))
            for step in range(NH // 2 + 1):
                pso_t = None
                if step >= 1:
                    pso_t = [pso_pool.tile([P, 512], F32, name=f"pso{i}", tag="pso")
                             for i in range(4)]
                for mt in range(MT):
                    if step < NH // 2:
                        for nt in range(2):
                            psq = psq_pool.tile([P, 1024], F32, name="psq", tag="psq")
                            for hh in range(2):
                                nc.tensor.matmul(
                                    psq[:, hh * 512:(hh + 1) * 512],
                                    qk_sb[4 + step][hh * HD:(hh + 1) * HD, mt * P:(mt + 1) * P],
                                    qk_sb[step][hh * HD:(hh + 1) * HD, nt * 512:(nt + 1) * 512],
                                    start=True, stop=True,
                                )
                            pt = ppool.tile([P, 1024], BF, name="ptile", tag="ptile")
                            nc.scalar.activation(
                                pt[:], psq[:], mybir.ActivationFunctionType.Exp,
                                scale=float(1.0 / np.sqrt(HD)),
                            )
                            p_tiles[(step, mt, nt)] = pt
                    if step >= 1:
                        pp = step - 1
                        for idx, (hh, nt) in enumerate(DRAIN_ORDER):
                            h = 2 * pp + hh
                            nc.tensor.matmul(
                                pso_t[idx][0:HD + 1, :],
                                v_sb[mt][:, h * (HD + 1):(h + 1) * (HD + 1)],
                                p_tiles[(pp, mt, nt)][:, hh * 512:(hh + 1) * 512],
                                start=(mt == 0), stop=(mt == MT - 1),
                            )
                if step >= 1:
                    pp = step - 1
                    for idx, (hh, nt) in enumerate(DRAIN_ORDER):
                        h = 2 * pp + hh
                        ct, half = h // 2, h % 2
                        rrow = mpool.tile([1, 512], F32, name="rrow", tag="rrow")
                        nc.vector.tensor_copy(rrow[0:1, :], pso_t[idx][HD:HD + 1, :])
                        rinv = mpool.tile([1, 512], F32, name="rinv", tag="rinv")
                        nc.vector.reciprocal_approx_fast(rinv[0:1, :], rrow[0:1, :])
                        bc = mpool.tile([HD, 512], F32, name="bc", tag="bc")
                        nc.gpsimd.partition_broadcast(bc[:], rinv[0:1, :], channels=HD)
                        nc.vector.tensor_mul(
                            oT_sb[ct][half * HD:(half + 1) * HD, nt * 512:(nt + 1) * 512],
                            pso_t[idx][0:HD, :], bc[:],
                        )

            # ---- output projection: yT[o, s] ----
            y_sb = [ypool.tile([P, S], F32, name=f"y{ot}", tag=f"y{ot}") for ot in range(CT)]
            og = [(ot, st_) for ot in range(CT) for st_ in range(2)]
            for bk in range(0, len(og), 4):
                blk = og[bk:bk + 4]
                pss = [pso_pool.tile([P, 512], F32, name=f"op{i}", tag="pso")
                       for i in range(len(blk))]
                for ct in range(CT):
                    for g, (ot, st_) in enumerate(blk):
                        nc.tensor.matmul(
                            pss[g][:],
                            wo_sb[ct][:, ot * P:(ot + 1) * P],
                            oT_sb[ct][:, st_ * 512:(st_ + 1) * 512],
                            start=(ct == 0), stop=(ct == CT - 1),
                        )
                for g, (ot, st_) in enumerate(blk):
                    dst = y_sb[ot][:, st_ * 512:(st_ + 1) * 512]
                    if g % 2 == 0:
                        nc.scalar.copy(dst, pss[g][:])
                    else:
                        nc.vector.tensor_copy(dst, pss[g][:])
            for ot in range(CT):
                nc.sync.dma_start(y_d[ot * P:(ot + 1) * P, :], y_sb[ot][:])

    nc.compile()
    return nc


def get_program():
    if "nc" not in _cache:
        _cache["nc"] = build_program()
    return _cache["nc"]


def kernel(x, w_qkv, w_out, b_out, _trace=False, _tmpdir=None):
    x = np.asarray(x, dtype=np.float32)
    w_qkv = np.asarray(w_qkv, dtype=np.float32)
    w_out = np.asarray(w_out, dtype=np.float32)
    b_out = np.asarray(b_out, dtype=np.float32)
    N = x.shape[0]

    xb = x.reshape(N, C, S).astype(ml_dtypes.bfloat16)
    wqT = np.ascontiguousarray(w_qkv.T).astype(ml_dtypes.bfloat16)
    woT = np.ascontiguousarray(w_out.T).astype(ml_dtypes.bfloat16)

    nc = get_program()
    in_maps = [
        {"x": np.ascontiguousarray(xb[n]), "wq": wqT, "wo": woT}
        for n in range(N)
    ]
    res = run_bass_kernel_spmd(
        nc, in_maps, core_ids=list(range(N)), trace=_trace, tmpdir=_tmpdir
    )
    y = np.stack([res.results[n]["y"] for n in range(N)])
    y = y.reshape(N, C, 32, 32).astype(np.float32)
    y = y + b_out[None, :, None, None]
    if _trace:
        return y, res
    return y


# revision 10
# speedup vs baseline: 1.7647x; 1.0084x over previous
"""MHA kernel for TRN2: x[8,512,32,32], 8 heads, S=1024, C=512.

Sharding: data-parallel over batch N=8 -> one batch item per NeuronCore.
Per-core layout (all transpose-free):
  qkT[e,s]  = w_qkvT[:, :1024].T @ x      (e on partitions; q tiles 0-3, k tiles 4-7)
  v[s,e]    = x.T @ w_qkvT[:, 1024:]      (s on partitions, natural layout)
  scoresT   = kT_h.T @ qT_h               (k_s on partitions; K=64 -> head pair packed
                                           at array rows 0-63 / 64-127)
  P         = exp(scoresT * 1/8)          (ACT, batched 2048-wide from PSUM)
  oT_aug    = [v_h | 1].T @ P             (M=65; row 64 = softmax denominator r)
  oT        = oT_aug[:64] * (1/r)         (gpsimd partition_broadcast of 1/r)
  yT[o,s]   = w_outT.T @ oT               (+ b_out added host-side; == NCHW layout)
"""

import numpy as np
import ml_dtypes

import concourse.bacc as bacc
import concourse.mybir as mybir
import concourse.tile as tile
from concourse.bass_utils import run_bass_kernel_spmd

P = 128
S = 1024          # sequence = 32*32
C = 512           # channels
NH = 8            # heads
HD = 64           # head dim
CT = C // P       # 4 c-tiles
ET = 2 * C // P   # 8 e-tiles for q+k
MT = S // P       # 8 s-tiles
BF = mybir.dt.bfloat16
F32 = mybir.dt.float32

_cache = {}


def build_program(dbg=False):
    nc = bacc.Bacc("TRN2", target_bir_lowering=False, debug=False, num_devices=8)
    x_d = nc.dram_tensor("x", [C, S], BF, kind="ExternalInput").ap()
    wq_d = nc.dram_tensor("wq", [C, 3 * C], BF, kind="ExternalInput").ap()
    wo_d = nc.dram_tensor("wo", [C, C], BF, kind="ExternalInput").ap()
    y_d = nc.dram_tensor("y", [C, S], F32, kind="ExternalOutput").ap()
    if dbg:
        dbg_qk0 = nc.dram_tensor("dbg_qk0", [P, S], BF, kind="ExternalOutput").ap()
        dbg_qk4 = nc.dram_tensor("dbg_qk4", [P, S], BF, kind="ExternalOutput").ap()
        dbg_v0 = nc.dram_tensor("dbg_v0", [P, NH * (HD + 1)], BF, kind="ExternalOutput").ap()
        dbg_p00 = nc.dram_tensor("dbg_p00", [P, 2048], BF, kind="ExternalOutput").ap()
        dbg_bc = nc.dram_tensor("dbg_bc", [HD, 512], F32, kind="ExternalOutput").ap()
        dbg_o0 = nc.dram_tensor("dbg_o0", [P, S], BF, kind="ExternalOutput").ap()

    with tile.TileContext(nc) as tc:
        with (
            tc.tile_pool(name="const", bufs=1) as cpool,
            tc.tile_pool(name="qk", bufs=1) as qkpool,
            tc.tile_pool(name="vp", bufs=1) as vpool,
            tc.tile_pool(name="pp", bufs=32) as ppool,
            tc.tile_pool(name="ot", bufs=1) as opool,
            tc.tile_pool(name="yp", bufs=1) as ypool,
            tc.tile_pool(name="misc", bufs=4) as mpool,
            tc.tile_pool(name="psq", bufs=2, space="PSUM") as psq_pool,
            tc.tile_pool(name="pso", bufs=4, space="PSUM") as pso_pool,
        ):
            # ---- load inputs ----
            x_sb, w_sb, wo_sb = [], [], []
            for ct in range(CT):
                xt = cpool.tile([P, S], BF, name=f"x{ct}", tag=f"x{ct}")
                nc.sync.dma_start(xt[:], x_d[ct * P:(ct + 1) * P, :])
                x_sb.append(xt)
                wt = cpool.tile([P, 3 * C], BF, name=f"w{ct}", tag=f"w{ct}")
                nc.sync.dma_start(wt[:, 0:2 * C], wq_d[ct * P:(ct + 1) * P, 0:2 * C])
                w_sb.append(wt)
            for ct in range(CT):
                nc.sync.dma_start(
                    w_sb[ct][:, 2 * C:3 * C], wq_d[ct * P:(ct + 1) * P, 2 * C:3 * C]
                )
            for ct in range(CT):
                wot = cpool.tile([P, C], BF, name=f"wo{ct}", tag=f"wo{ct}")
                nc.sync.dma_start(wot[:], wo_d[ct * P:(ct + 1) * P, :])
                wo_sb.append(wot)

            # ---- qkT projection: [e=1024 rows, s=1024] ----
            qk_sb = []
            for et in range(ET):
                t = qkpool.tile([P, S], BF, name=f"qk{et}", tag=f"qk{et}")
                qk_sb.append(t)
            groups = [(et, nt) for et in range(ET) for nt in range(2)]
            for bk in range(0, len(groups), 4):
                blk = groups[bk:bk + 4]
                pss = [pso_pool.tile([P, 512], F32, name=f"qp{i}", tag="pso")
                       for i in range(len(blk))]
                for ct in range(CT):
                    for g, (et, nt) in enumerate(blk):
                        nc.tensor.matmul(
                            pss[g][:],
                            w_sb[ct][:, et * P:(et + 1) * P],
                            x_sb[ct][:, nt * 512:(nt + 1) * 512],
                            start=(ct == 0), stop=(ct == CT - 1),
                        )
                for g, (et, nt) in enumerate(blk):
                    dst = qk_sb[et][:, nt * 512:(nt + 1) * 512]
                    nc.vector.tensor_copy(dst, pss[g][:])

            # ---- v projection, natural layout + ones col: [s, 8*(64+1)] ----
            v_sb = [None] * MT
            for bk in range(0, MT, 4):
                pss = [pso_pool.tile([P, 512], F32, name=f"vp{i}", tag="pso")
                       for i in range(4)]
                for ct in range(CT):
                    for g in range(4):
                        mt = bk + g
                        nc.tensor.matmul(
                            pss[g][:],
                            x_sb[ct][:, mt * P:(mt + 1) * P],
                            w_sb[ct][:, 2 * C:3 * C],
                            start=(ct == 0), stop=(ct == CT - 1),
                        )
                for g in range(4):
                    mt = bk + g
                    vt = vpool.tile([P, NH * (HD + 1)], BF, name=f"v{mt}", tag=f"v{mt}")
                    nc.vector.memset(vt[:], 1.0)
                    dst = vt[:].rearrange("p (h e) -> p h e", e=HD + 1)[:, :, 0:HD]
                    srcp = pss[g][:].rearrange("p (h e) -> p h e", e=HD)
                    nc.vector.tensor_copy(dst, srcp)
                    v_sb[mt] = vt

            # ---- attention, software-pipelined: QK/exp(pair p) || PV(pair p-1) ----
            oT_sb = [opool.tile([P, S], BF, name=f"o{ct}", tag=f"o{ct}") for ct in range(CT)]
            p_tiles = {}
            DRAIN_ORDER = ((0, 0), (1, 0), (0, 1), (1, 1))
            for step in range(NH // 2 + 1):
                pso_t = None
                if step >= 1:
                    pso_t = [pso_pool.tile([P, 512], F32, name=f"pso{i}", tag="pso")
                             for i in range(4)]
                for mt in range(MT):
                    if step < NH // 2:
                        for nt in range(2):
                            psq = psq_pool.tile([P, 1024], F32, name="psq", tag="psq")
                            for hh in range(2):
                                nc.tensor.matmul(
                                    psq[:, hh * 512:(hh + 1) * 512],
                                    qk_sb[4 + step][hh * HD:(hh + 1) * HD, mt * P:(mt + 1) * P],
                                    qk_sb[step][hh * HD:(hh + 1) * HD, nt * 512:(nt + 1) * 512],
                                    start=True, stop=True,
                                )
                            pt = ppool.tile([P, 1024], BF, name="ptile", tag="ptile")
                            nc.scalar.activation(
                                pt[:], psq[:], mybir.ActivationFunctionType.Exp,
                                scale=float(1.0 / np.sqrt(HD)),
                            )
                            p_tiles[(step, mt, nt)] = pt
                    if step >= 1:
                        pp = step - 1
                        for idx, (hh, nt) in enumerate(DRAIN_ORDER):
                            h = 2 * pp + hh
                            nc.tensor.matmul(
                                pso_t[idx][0:HD + 1, :],
                                v_sb[mt][:, h * (HD + 1):(h + 1) * (HD + 1)],
                                p_tiles[(pp, mt, nt)][:, hh * 512:(hh + 1) * 512],
                                start=(mt == 0), stop=(mt == MT - 1),
                            )
                if step >= 1:
                    pp = step - 1
                    for idx, (hh, nt) in enumerate(DRAIN_ORDER):
                        h = 2 * pp + hh
                        ct, half = h // 2, h % 2
                        rrow = mpool.tile([1, 512], F32, name="rrow", tag="rrow")
                        nc.vector.tensor_copy(rrow[0:1, :], pso_t[idx][HD:HD + 1, :])
                        rinv = mpool.tile([1, 512], F32, name="rinv", tag="rinv")
                        nc.vector.reciprocal_approx_fast(rinv[0:1, :], rrow[0:1, :])
                        bc = mpool.tile([HD, 512], F32, name="bc", tag="bc")
                        nc.gpsimd.partition_broadcast(bc[:], rinv[0:1, :], channels=HD)
                        nc.vector.tensor_mul(
                            oT_sb[ct][half * HD:(half + 1) * HD, nt * 512:(nt + 1) * 512],
                            pso_t[idx][0:HD, :], bc[:],
                        )

            # ---- output projection: yT[o, s] ----
            y_sb = [ypool.tile([P, S], F32, name=f"y{ot}", tag=f"y{ot}") for ot in range(CT)]
            og = [(ot, st_) for ot in range(CT) for st_ in range(2)]
            for bk in range(0, len(og), 4):
                blk = og[bk:bk + 4]
                pss = [pso_pool.tile([P, 512], F32, name=f"op{i}", tag="pso")
                       for i in range(len(blk))]
                for ct in range(CT):
                    for g, (ot, st_) in enumerate(blk):
                        nc.tensor.matmul(
                            pss[g][:],
                            wo_sb[ct][:, ot * P:(ot + 1) * P],
                            oT_sb[ct][:, st_ * 512:(st_ + 1) * 512],
                            start=(ct == 0), stop=(ct == CT - 1),
                        )
                for g, (ot, st_) in enumerate(blk):
                    dst = y_sb[ot][:, st_ * 512:(st_ + 1) * 512]
                    nc.vector.tensor_copy(dst, pss[g][:])
                    nc.sync.dma_start(
                        y_d[ot * P:(ot + 1) * P, st_ * 512:(st_ + 1) * 512], dst
                    )

    nc.compile()
    return nc


def get_program():
    if "nc" not in _cache:
        _cache["nc"] = build_program()
    return _cache["nc"]


def kernel(x, w_qkv, w_out, b_out, _trace=False, _tmpdir=None):
    x = np.asarray(x, dtype=np.float32)
    w_qkv = np.asarray(w_qkv, dtype=np.float32)
    w_out = np.asarray(w_out, dtype=np.float32)
    b_out = np.asarray(b_out, dtype=np.float32)
    N = x.shape[0]

    xb = x.reshape(N, C, S).astype(ml_dtypes.bfloat16)
    wqT = np.ascontiguousarray(w_qkv.T).astype(ml_dtypes.bfloat16)
    woT = np.ascontiguousarray(w_out.T).astype(ml_dtypes.bfloat16)

    nc = get_program()
    in_maps = [
        {"x": np.ascontiguousarray(xb[n]), "wq": wqT, "wo": woT}
        for n in range(N)
    ]
    res = run_bass_kernel_spmd(
        nc, in_maps, core_ids=list(range(N)), trace=_trace, tmpdir=_tmpdir
    )
    y = np.stack([res.results[n]["y"] for n in range(N)])
    y = y.reshape(N, C, 32, 32).astype(np.float32)
    y = y + b_out[None, :, None, None]
    if _trace:
        return y, res
    return y


# revision 11
# speedup vs baseline: 1.8479x; 1.0472x over previous
"""MHA kernel for TRN2: x[8,512,32,32], 8 heads, S=1024, C=512.

Sharding: data-parallel over batch N=8 -> one batch item per NeuronCore.
Per-core layout (all transpose-free):
  qkT[e,s]  = w_qkvT[:, :1024].T @ x      (e on partitions; q tiles 0-3, k tiles 4-7)
  v[s,e]    = x.T @ w_qkvT[:, 1024:]      (s on partitions, natural layout)
  scoresT   = kT_h.T @ qT_h               (k_s on partitions; K=64 -> head pair packed
                                           at array rows 0-63 / 64-127)
  P         = exp(scoresT * 1/8)          (ACT, batched 2048-wide from PSUM)
  oT_aug    = [v_h | 1].T @ P             (M=65; row 64 = softmax denominator r)
  oT        = oT_aug[:64] * (1/r)         (gpsimd partition_broadcast of 1/r)
  yT[o,s]   = w_outT.T @ oT               (+ b_out added host-side; == NCHW layout)
"""

import numpy as np
import ml_dtypes

import concourse.bacc as bacc
import concourse.mybir as mybir
import concourse.tile as tile
from concourse.bass_utils import run_bass_kernel_spmd

P = 128
S = 1024          # sequence = 32*32
C = 512           # channels
NH = 8            # heads
HD = 64           # head dim
CT = C // P       # 4 c-tiles
ET = 2 * C // P   # 8 e-tiles for q+k
MT = S // P       # 8 s-tiles
BF = mybir.dt.bfloat16
F32 = mybir.dt.float32

_cache = {}


def build_program(dbg=False):
    nc = bacc.Bacc("TRN2", target_bir_lowering=False, debug=False, num_devices=8)
    x_d = nc.dram_tensor("x", [C, S], BF, kind="ExternalInput").ap()
    wq_d = nc.dram_tensor("wq", [C, 3 * C], BF, kind="ExternalInput").ap()
    wo_d = nc.dram_tensor("wo", [C, C], BF, kind="ExternalInput").ap()
    y_d = nc.dram_tensor("y", [C, S], F32, kind="ExternalOutput").ap()
    if dbg:
        dbg_qk0 = nc.dram_tensor("dbg_qk0", [P, S], BF, kind="ExternalOutput").ap()
        dbg_qk4 = nc.dram_tensor("dbg_qk4", [P, S], BF, kind="ExternalOutput").ap()
        dbg_v0 = nc.dram_tensor("dbg_v0", [P, NH * (HD + 1)], BF, kind="ExternalOutput").ap()
        dbg_p00 = nc.dram_tensor("dbg_p00", [P, 2048], BF, kind="ExternalOutput").ap()
        dbg_bc = nc.dram_tensor("dbg_bc", [HD, 512], F32, kind="ExternalOutput").ap()
        dbg_o0 = nc.dram_tensor("dbg_o0", [P, S], BF, kind="ExternalOutput").ap()

    with tile.TileContext(nc) as tc:
        with (
            tc.tile_pool(name="const", bufs=1) as cpool,
            tc.tile_pool(name="qk", bufs=1) as qkpool,
            tc.tile_pool(name="vp", bufs=1) as vpool,
            tc.tile_pool(name="pp", bufs=32) as ppool,
            tc.tile_pool(name="ot", bufs=1) as opool,
            tc.tile_pool(name="yp", bufs=1) as ypool,
            tc.tile_pool(name="misc", bufs=4) as mpool,
            tc.tile_pool(name="psq", bufs=2, space="PSUM") as psq_pool,
            tc.tile_pool(name="pso", bufs=4, space="PSUM") as pso_pool,
        ):
            # ---- load inputs ----
            x_sb, w_sb, wo_sb = [], [], []
            for ct in range(CT):
                xt = cpool.tile([P, S], BF, name=f"x{ct}", tag=f"x{ct}")
                nc.sync.dma_start(xt[:], x_d[ct * P:(ct + 1) * P, :])
                x_sb.append(xt)
                wt = cpool.tile([P, 3 * C], BF, name=f"w{ct}", tag=f"w{ct}")
                nc.sync.dma_start(wt[:, 0:2 * C], wq_d[ct * P:(ct + 1) * P, 0:2 * C])
                w_sb.append(wt)
            for ct in range(CT):
                nc.sync.dma_start(
                    w_sb[ct][:, 2 * C:3 * C], wq_d[ct * P:(ct + 1) * P, 2 * C:3 * C]
                )
            for ct in range(CT):
                wot = cpool.tile([P, C], BF, name=f"wo{ct}", tag=f"wo{ct}")
                nc.sync.dma_start(wot[:], wo_d[ct * P:(ct + 1) * P, :])
                wo_sb.append(wot)

            # ---- qkT projection: [e=1024 rows, s=1024] ----
            qk_sb = []
            for et in range(ET):
                t = qkpool.tile([P, S], BF, name=f"qk{et}", tag=f"qk{et}")
                qk_sb.append(t)
            v_sb = [None] * MT

            def emit_qkv_group(et, nt):
                ps = pso_pool.tile([P, 512], F32, name="qp", tag="pso")
                for ct in range(CT):
                    nc.tensor.matmul(
                        ps[:],
                        w_sb[ct][:, et * P:(et + 1) * P],
                        x_sb[ct][:, nt * 512:(nt + 1) * 512],
                        start=(ct == 0), stop=(ct == CT - 1),
                    )
                nc.vector.tensor_copy(qk_sb[et][:, nt * 512:(nt + 1) * 512], ps[:])

            def emit_v_group(mt):
                ps = pso_pool.tile([P, 512], F32, name="vp", tag="pso")
                for ct in range(CT):
                    nc.tensor.matmul(
                        ps[:],
                        x_sb[ct][:, mt * P:(mt + 1) * P],
                        w_sb[ct][:, 2 * C:3 * C],
                        start=(ct == 0), stop=(ct == CT - 1),
                    )
                vt = vpool.tile([P, NH * (HD + 1)], BF, name=f"v{mt}", tag=f"v{mt}")
                nc.vector.memset(vt[:], 1.0)
                dst = vt[:].rearrange("p (h e) -> p h e", e=HD + 1)[:, :, 0:HD]
                nc.vector.tensor_copy(dst, ps[:].rearrange("p (h e) -> p h e", e=HD))
                v_sb[mt] = vt

            # block A: the tiles pair-0 attention needs first
            for et, nt in ((0, 0), (4, 0), (0, 1), (4, 1)):
                emit_qkv_group(et, nt)
            pending = [("qkv", et, nt) for et in (1, 5, 2, 6, 3, 7) for nt in (0, 1)]
            pending += [("v", mt, None) for mt in range(MT)]
            pend_i = 0

            # ---- attention, software-pipelined: QK/exp(pair p) || PV(pair p-1);
            #      step 0 also drains the remaining qkv/v projection groups ----
            oT_sb = [opool.tile([P, S], BF, name=f"o{ct}", tag=f"o{ct}") for ct in range(CT)]
            p_tiles = {}
            DRAIN_ORDER = ((0, 0), (1, 0), (0, 1), (1, 1))
            for step in range(NH // 2 + 1):
                pso_t = None
                if step >= 1:
                    pso_t = [pso_pool.tile([P, 512], F32, name=f"pso{i}", tag="pso")
                             for i in range(4)]
                for mt in range(MT):
                    if step < NH // 2:
                        for nt in range(2):
                            psq = psq_pool.tile([P, 1024], F32, name="psq", tag="psq")
                            for hh in range(2):
                                nc.tensor.matmul(
                                    psq[:, hh * 512:(hh + 1) * 512],
                                    qk_sb[4 + step][hh * HD:(hh + 1) * HD, mt * P:(mt + 1) * P],
                                    qk_sb[step][hh * HD:(hh + 1) * HD, nt * 512:(nt + 1) * 512],
                                    start=True, stop=True,
                                )
                            pt = ppool.tile([P, 1024], BF, name="ptile", tag="ptile")
                            nc.scalar.activation(
                                pt[:], psq[:], mybir.ActivationFunctionType.Exp,
                                scale=float(1.0 / np.sqrt(HD)),
                            )
                            p_tiles[(step, mt, nt)] = pt
                            if step == 0:
                                slot = mt * 2 + nt
                                want = 20 * (slot + 1) // 16
                                while pend_i < min(want, 20):
                                    kind, i1, i2 = pending[pend_i]
                                    if kind == "qkv":
                                        emit_qkv_group(i1, i2)
                                    else:
                                        emit_v_group(i1)
                                    pend_i += 1
                    if step >= 1:
                        pp = step - 1
                        for idx, (hh, nt) in enumerate(DRAIN_ORDER):
                            h = 2 * pp + hh
                            nc.tensor.matmul(
                                pso_t[idx][0:HD + 1, :],
                                v_sb[mt][:, h * (HD + 1):(h + 1) * (HD + 1)],
                                p_tiles[(pp, mt, nt)][:, hh * 512:(hh + 1) * 512],
                                start=(mt == 0), stop=(mt == MT - 1),
                            )
                if step >= 1:
                    pp = step - 1
                    for idx, (hh, nt) in enumerate(DRAIN_ORDER):
                        h = 2 * pp + hh
                        ct, half = h // 2, h % 2
                        rrow = mpool.tile([1, 512], F32, name="rrow", tag="rrow")
                        nc.vector.tensor_copy(rrow[0:1, :], pso_t[idx][HD:HD + 1, :])
                        rinv = mpool.tile([1, 512], F32, name="rinv", tag="rinv")
                        nc.vector.reciprocal_approx_fast(rinv[0:1, :], rrow[0:1, :])
                        bc = mpool.tile([HD, 512], F32, name="bc", tag="bc")
                        nc.gpsimd.partition_broadcast(bc[:], rinv[0:1, :], channels=HD)
                        nc.vector.tensor_mul(
                            oT_sb[ct][half * HD:(half + 1) * HD, nt * 512:(nt + 1) * 512],
                            pso_t[idx][0:HD, :], bc[:],
                        )

            # ---- output projection: yT[o, s] ----
            y_sb = [ypool.tile([P, S], F32, name=f"y{ot}", tag=f"y{ot}") for ot in range(CT)]
            og = [(ot, st_) for ot in range(CT) for st_ in range(2)]
            for bk in range(0, len(og), 4):
                blk = og[bk:bk + 4]
                pss = [pso_pool.tile([P, 512], F32, name=f"op{i}", tag="pso")
                       for i in range(len(blk))]
                for ct in range(CT):
                    for g, (ot, st_) in enumerate(blk):
                        nc.tensor.matmul(
                            pss[g][:],
                            wo_sb[ct][:, ot * P:(ot + 1) * P],
                            oT_sb[ct][:, st_ * 512:(st_ + 1) * 512],
                            start=(ct == 0), stop=(ct == CT - 1),
                        )
                for g, (ot, st_) in enumerate(blk):
                    dst = y_sb[ot][:, st_ * 512:(st_ + 1) * 512]
                    nc.vector.tensor_copy(dst, pss[g][:])
                    nc.sync.dma_start(
                        y_d[ot * P:(ot + 1) * P, st_ * 512:(st_ + 1) * 512], dst
                    )

    nc.compile()
    return nc


def get_program():
    if "nc" not in _cache:
        _cache["nc"] = build_program()
    return _cache["nc"]


def kernel(x, w_qkv, w_out, b_out, _trace=False, _tmpdir=None):
    x = np.asarray(x, dtype=np.float32)
    w_qkv = np.asarray(w_qkv, dtype=np.float32)
    w_out = np.asarray(w_out, dtype=np.float32)
    b_out = np.asarray(b_out, dtype=np.float32)
    N = x.shape[0]

    xb = x.reshape(N, C, S).astype(ml_dtypes.bfloat16)
    wqT = np.ascontiguousarray(w_qkv.T).astype(ml_dtypes.bfloat16)
    woT = np.ascontiguousarray(w_out.T).astype(ml_dtypes.bfloat16)

    nc = get_program()
    in_maps = [
        {"x": np.ascontiguousarray(xb[n]), "wq": wqT, "wo": woT}
        for n in range(N)
    ]
    res = run_bass_kernel_spmd(
        nc, in_maps, core_ids=list(range(N)), trace=_trace, tmpdir=_tmpdir
    )
    y = np.stack([res.results[n]["y"] for n in range(N)])
    y = y.reshape(N, C, 32, 32).astype(np.float32)
    y = y + b_out[None, :, None, None]
    if _trace:
        return y, res
    return y
